# revision 1
# baseline (speedup 1.0000x reference)
"""AttnBlockWithText Trainium2 Bass kernel.

Math (per batch element b, fully data-parallel over 8 NeuronCores):
  h   = concat([x_b, broadcast(text_b)])            # [768, 1024]
  hn  = GroupNorm(32, 768, eps=1e-6)(h) * gamma + beta
  q   = W0^T hn + b0 ; k = W1^T hn + b1 ; v = W2^T hn + b2
  4-head attention over the 1024 spatial positions, out = x + atten(q,k,v)

Key restructurings (validated bit-exact vs reference in fp64):
  * The text channels are spatially constant, so they are never
    materialized: their GroupNorm statistics are analytic (1024*t,
    1024*t^2 folded into the group-indicator matmul), and their QKV
    contribution is a per-channel constant folded into bias terms.
  * k's bias (b1 + text part) is skipped entirely: adding a constant
    vector to k shifts every score row by a query-dependent constant,
    which softmax cancels.
  * Scores are computed key-major (S_T = k^T q) so the [1024, 1024]
    probability matrix never needs a transpose for the AV matmul; v is
    produced directly transposed (vT = hn^T W2).
  * Softmax max-subtraction is skipped (|S| <= ~20 -> exp is safe in
    fp32) and the denominator comes free from a ones-column appended to
    vT in the AV matmul (M=65).
  * Large matmuls run in float32r (TF32-like, 1 cycle/row at free-dim
    >= 256, ~1.6e-4 rel err) accumulated in fp32 PSUM.
  * Softmax division: per-pair denominator rows are gathered to
    partition 0 with a small DMA, inverted with the fast Newton
    reciprocal (multi-partition, base 0 -- the custom DVE op and the
    GPSIMD partition broadcast both require base-0 operands on HW),
    broadcast across partitions on the otherwise-idle GPSIMD, then a
    DVE multiply + residual add finish each head.
"""

import sys

sys.path.insert(0, "/opt/trn_rl_repo")

import numpy as np

import concourse.bass as bass
import concourse.mybir as mybir
import concourse.tile as tile
from concourse import bacc
from concourse.bass_utils import run_bass_kernel_spmd

F32 = mybir.dt.float32
F32R = mybir.dt.float32r
AF = mybir.ActivationFunctionType
OP = mybir.AluOpType
AX = mybir.AxisListType

C = 256          # x channels
TC = 512         # text channels
CIN = C + TC     # 768
HW = 1024        # 32*32 spatial
NH = 4           # heads
NG = 32          # groupnorm groups
CPG = CIN // NG  # 24 channels per group
EPS = 1e-6
INV_CNT = 1.0 / (CPG * HW)

_PROGRAM = None
_last_in_maps = None


def _build_program():
    nc = bacc.Bacc(None, target_bir_lowering=False)

    x_d = nc.dram_tensor("x", [C, HW], F32, kind="ExternalInput")
    # packed small inputs: tcol[0:4] gam[4:10] bet[10:16] bias0[16:18]
    misc_d = nc.dram_tensor("misc", [128, 18], F32, kind="ExternalInput")
    b2r_d = nc.dram_tensor("b2row", [1, C], F32, kind="ExternalInput")
    gmat_d = nc.dram_tensor("gmat", [128, 6 * NG], F32, kind="ExternalInput")
    emat_d = nc.dram_tensor("emat", [NG, CIN], F32, kind="ExternalInput")
    # wall: [128, 2*3*256] f32r -- kc-major, then (W0,W1,W2)
    wall_d = nc.dram_tensor("wall", [128, 1536], F32R, kind="ExternalInput")
    # wtext: [128, 4*(256+256)] f32 -- kc-major, then (W0t, W2t)
    wtext_d = nc.dram_tensor("wtext", [128, 2048], F32, kind="ExternalInput")
    out_d = nc.dram_tensor("out", [C, HW], F32, kind="ExternalOutput")

    with tile.TileContext(nc) as tc:
        with tc.tile_pool(name="sb", bufs=1) as pool:
            # ---------------- persistent inputs (9 DMAs) ----------------
            x_sb = []
            for m in range(2):
                xt = pool.tile([128, HW], F32, name=f"x{m}")
                for p in range(2):
                    nc.sync.dma_start(
                        xt[64 * p:64 * (p + 1), :],
                        x_d.ap()[128 * m + 64 * p:128 * m + 64 * (p + 1), :])
                x_sb.append(xt)
            misc = pool.tile([128, 18], F32, name="misc_sb")
            nc.sync.dma_start(misc, misc_d.ap())
            gm = pool.tile([128, 6 * NG], F32, name="gm_sb")
            nc.sync.dma_start(gm, gmat_d.ap())
            em = pool.tile([NG, CIN], F32, name="em_sb")
            nc.sync.dma_start(em, emat_d.ap())
            wall = pool.tile([128, 1536], F32R, name="wall_sb")
            nc.sync.dma_start(wall, wall_d.ap())
            wtext = pool.tile([128, 2048], F32, name="wtext_sb")
            nc.sync.dma_start(wtext, wtext_d.ap())
            b2r = pool.tile([1, C], F32, name="b2r_sb")
            nc.sync.dma_start(b2r, b2r_d.ap())
            # per-head copy of x (base-partition-0 aligned for the finals)
            xh4 = pool.tile([64, 4 * HW], F32, name="xh4")
            nc.sync.dma_start(
                xh4.rearrange("p (h w) -> p h w", w=HW),
                x_d.ap().rearrange("(h p) w -> p h w", p=64))

            tcol = misc[:, 0:4]
            gam6 = misc[:, 4:10]
            bet6 = misc[:, 10:16]
            bias0 = misc[:, 16:18]
            wq = [wall[:, 768 * kc + 0:768 * kc + 256] for kc in range(2)]
            wk = [wall[:, 768 * kc + 256:768 * kc + 512] for kc in range(2)]
            wv = [wall[:, 768 * kc + 512:768 * kc + 768] for kc in range(2)]
            w0t = [wtext[:, 512 * kc:512 * kc + 256] for kc in range(4)]
            w2t = [wtext[:, 512 * kc + 256:512 * kc + 512] for kc in range(4)]

            ones_f = pool.tile([128, 4], F32, name="ones_f")
            nc.vector.memset(ones_f, 1.0)
            ones_r = pool.tile([128, 4], F32R, name="ones_r")
            nc.vector.tensor_copy(ones_r, ones_f)

            with tc.tile_pool(name="ps1", bufs=1, space="PSUM") as ps1:
                # ---------------- group statistics ----------------
                st = []
                for cc in range(2):
                    stt = pool.tile([128, 2], F32, name=f"st{cc}")
                    scratch = pool.tile([128, HW], F32, tag="scr", bufs=2,
                                        name=f"scr{cc}")
                    # sum(x^2) on ScalarE (idle at startup), sum(x) on DVE;
                    # per partition-half so each starts as its DMA lands
                    for p in range(2):
                        sl = slice(64 * p, 64 * (p + 1))
                        nc.scalar.activation(scratch[sl, :], x_sb[cc][sl, :],
                                             AF.Square,
                                             accum_out=stt[sl, 1:2])
                        nc.vector.reduce_sum(stt[sl, 0:1], x_sb[cc][sl, :],
                                             axis=AX.X)
                    st.append(stt)
                for j in range(4):
                    stt = pool.tile([128, 2], F32, name=f"stt{j}")
                    nc.vector.tensor_copy(stt[:, 0:1], tcol[:, j:j + 1])
                    nc.vector.tensor_scalar(
                        out=stt[:, 1:2], in0=tcol[:, j:j + 1],
                        scalar1=tcol[:, j:j + 1], scalar2=None, op0=OP.mult)
                    st.append(stt)

                ps_st = ps1.tile([NG, 2], F32, tag="sps", bufs=2,
                                 name="ps_st")
                for cc in range(6):
                    nc.tensor.matmul(ps_st, gm[:, NG * cc:NG * (cc + 1)],
                                     st[cc], start=(cc == 0), stop=(cc == 5))

                sms = pool.tile([NG, 2], F32, name="sms")
                nc.vector.tensor_scalar(out=sms, in0=ps_st, scalar1=INV_CNT,
                                        scalar2=None, op0=OP.mult)
                mu = sms[:, 0:1]
                m2 = sms[:, 1:2]
                nvar = pool.tile([NG, 1], F32, name="nvar")
                nc.vector.scalar_tensor_tensor(out=nvar, in0=mu, scalar=mu,
                                               in1=m2, op0=OP.mult,
                                               op1=OP.subtract)
                veps = pool.tile([NG, 1], F32, name="veps")
                nc.vector.tensor_scalar(out=veps, in0=nvar, scalar1=-1.0,
                                        scalar2=EPS, op0=OP.mult, op1=OP.add)
                # rsqrt: linear seed + 3 Newton steps (var is ~1 for
                # normalized inputs; exact to ~1e-6 for var in [0.4, 2.5])
                ya = pool.tile([NG, 1], F32, name="ya")
                yb = pool.tile([NG, 1], F32, name="yb")
                t2 = pool.tile([NG, 1], F32, name="t2c")
                uu = pool.tile([NG, 1], F32, name="uu")
                nc.vector.tensor_scalar(out=ya, in0=veps, scalar1=-0.5,
                                        scalar2=1.5, op0=OP.mult, op1=OP.add)
                cur, nxt = ya, yb
                for it in range(3):
                    nc.vector.tensor_scalar(out=t2, in0=veps, scalar1=cur,
                                            scalar2=cur, op0=OP.mult,
                                            op1=OP.mult)
                    nc.vector.tensor_scalar(out=uu, in0=t2, scalar1=-0.5,
                                            scalar2=1.5, op0=OP.mult,
                                            op1=OP.add)
                    dst = sms[:, 1:2] if it == 2 else nxt
                    nc.vector.tensor_scalar(out=dst, in0=cur, scalar1=uu,
                                            scalar2=None, op0=OP.mult)
                    cur, nxt = nxt, cur
                mr = sms

                # expand per-group (mu, rsqrt) to per-channel, all chunks in
                # one psum tile -> 3 vectorized DVE ops for scale/shift
                pse = ps1.tile([128, 12], F32, tag="sps", bufs=2, name="pse")
                for cc in range(6):
                    nc.tensor.matmul(pse[:, 2 * cc:2 * (cc + 1)],
                                     em[:, 128 * cc:128 * (cc + 1)],
                                     mr, start=True, stop=True)
                pse_mu = pse.rearrange("p (c two) -> p c two", two=2)[:, :, 0]
                pse_rs = pse.rearrange("p (c two) -> p c two", two=2)[:, :, 1]
                sc6 = pool.tile([128, 6], F32, name="sc6")
                nc.vector.tensor_tensor(out=sc6, in0=pse_rs, in1=gam6,
                                        op=OP.mult)
                mg6 = pool.tile([128, 6], F32, name="mg6")
                nc.vector.tensor_tensor(out=mg6, in0=pse_mu, in1=sc6,
                                        op=OP.mult)
                ngt6 = pool.tile([128, 6], F32, name="ngt6")
                nc.vector.tensor_tensor(out=ngt6, in0=mg6, in1=bet6,
                                        op=OP.subtract)  # = mu*s - beta

                # normalized x channels (f32r, ready as matmul operand)
                hn = []
                for cc in range(2):
                    hnt = pool.tile([128, HW], F32R, name=f"hn{cc}")
                    nc.vector.tensor_scalar(out=hnt, in0=x_sb[cc],
                                            scalar1=sc6[:, cc:cc + 1],
                                            scalar2=ngt6[:, cc:cc + 1],
                                            op0=OP.mult, op1=OP.subtract)
                    hn.append(hnt)
                # normalized text channels (constant over space): [128,1] x4
                hnt_cols = []
                for j in range(4):
                    ht = pool.tile([128, 1], F32, name=f"hnt{j}")
                    nc.vector.tensor_scalar(out=ht, in0=tcol[:, j:j + 1],
                                            scalar1=sc6[:, 2 + j:3 + j],
                                            scalar2=ngt6[:, 2 + j:3 + j],
                                            op0=OP.mult, op1=OP.subtract)
                    hnt_cols.append(ht)

                # q bias = W0t^T hn_t + b0 (per-channel col), v text row
                qb_cols = []
                for m in range(2):
                    psq = ps1.tile([128, 1], F32, tag="sps", bufs=2,
                                   name=f"psqb{m}")
                    for kc in range(4):
                        nc.tensor.matmul(
                            psq, w0t[kc][:, 128 * m:128 * (m + 1)],
                            hnt_cols[kc], start=(kc == 0), stop=(kc == 3))
                    qb = pool.tile([128, 1], F32, name=f"qb{m}")
                    nc.vector.tensor_scalar(out=qb, in0=psq,
                                            scalar1=bias0[:, m:m + 1],
                                            scalar2=None, op0=OP.add)
                    qb_cols.append(qb)
                ps_vtx = ps1.tile([1, C], F32, tag="sps", bufs=2,
                                  name="ps_vtx")
                for kc in range(4):
                    nc.tensor.matmul(ps_vtx, hnt_cols[kc], w2t[kc],
                                     start=(kc == 0), stop=(kc == 3))
                vtext = pool.tile([1, C], F32, name="vtext")
                nc.vector.tensor_tensor(out=vtext, in0=ps_vtx, in1=b2r,
                                        op=OP.add)
                # broadcast vtext over all partitions for the vT epilogue
                vtext_b = pool.tile([128, C], F32, name="vtext_b")
                nc.gpsimd.partition_broadcast(vtext_b, vtext)

                # ---------------- q, k projections ----------------
                q_sb, k_sb = [], []
                for m in range(2):
                    psq = ps1.tile([128, HW], F32, tag="qk", bufs=2,
                                   name=f"psq{m}")
                    for kc in range(2):
                        for n in range(2):
                            nc.tensor.matmul(
                                psq[:, 512 * n:512 * (n + 1)],
                                wq[kc][:, 128 * m:128 * (m + 1)],
                                hn[kc][:, 512 * n:512 * (n + 1)],
                                start=(kc == 0), stop=(kc == 1))
                    qt = pool.tile([128, HW], F32R, name=f"q{m}")
                    nc.scalar.activation(qt, psq, AF.Identity,
                                         bias=qb_cols[m], scale=1.0)
                    q_sb.append(qt)
                for m in range(2):
                    psk = ps1.tile([128, HW], F32, tag="qk", bufs=2,
                                   name=f"psk{m}")
                    for kc in range(2):
                        for n in range(2):
                            nc.tensor.matmul(
                                psk[:, 512 * n:512 * (n + 1)],
                                wk[kc][:, 128 * m:128 * (m + 1)],
                                hn[kc][:, 512 * n:512 * (n + 1)],
                                start=(kc == 0), stop=(kc == 1))
                    kt = pool.tile([128, HW], F32R, name=f"k{m}")
                    nc.vector.tensor_copy(kt, psk)
                    k_sb.append(kt)


            # ---------------- attention ----------------
            # Heads run sequentially: each head's softmax-division tail
            # (den row -> DMA gather -> fast reciprocal -> gpsimd
            # partition-broadcast -> DVE mult + residual) overlaps the next
            # head's exp stream, so only head 3's chain is exposed at the
            # end. vT chunks are emitted interleaved with head 0's score
            # loop so the PE in-order queue reaches the first scores
            # immediately; AV matmuls are deferred per head (they hide
            # under that head's last exps via the e-tile ring).
            with tc.tile_pool(name="ps2", bufs=1, space="PSUM") as ps2:
                vt_sb = [None] * 8
                e_all = {}
                for h in range(NH):
                    m, r = h // 2, h % 2
                    for i in range(8):      # key chunks
                        ss = ps2.tile([128, HW], F32, tag="sc", bufs=2,
                                      name=f"ss{h}{i}")
                        for n in range(2):
                            nc.tensor.matmul(
                                ss[:, 512 * n:512 * (n + 1)],
                                k_sb[m][64 * r:64 * (r + 1),
                                        128 * i:128 * (i + 1)],
                                q_sb[m][64 * r:64 * (r + 1),
                                        512 * n:512 * (n + 1)],
                                start=True, stop=True,
                                tile_position=(64 * r, 0))
                        et = pool.tile([128, HW], F32R, tag="e", bufs=12,
                                       name=f"e{h}{i}")
                        nc.scalar.activation(et, ss, AF.Exp, scale=0.125)
                        e_all[(h, i)] = et
                        if h == 0:
                            # vT = hn^T W2 + text row; layout [128, 4*66]:
                            # head hh data at cols 66hh..66hh+63, ones col
                            # at 66hh+64 (denominator column for AV)
                            psv = ps2.tile([128, C], F32, tag="av", bufs=2,
                                           name=f"psv{i}")
                            for kc in range(2):
                                nc.tensor.matmul(
                                    psv, hn[kc][:, 128 * i:128 * (i + 1)],
                                    wv[kc], start=(kc == 0), stop=(kc == 1))
                            vtt = pool.tile([128, 4 * 66], F32R,
                                            name=f"vt{i}")
                            dst = vtt.rearrange("p (hh c) -> p hh c",
                                                c=66)[:, :, 0:64]
                            src = psv.rearrange("p (hh c) -> p hh c", c=64)
                            vb = vtext_b.rearrange("p (hh c) -> p hh c",
                                                   c=64)
                            nc.vector.scalar_tensor_tensor(
                                out=dst, in0=src, scalar=1.0, in1=vb,
                                op0=OP.bypass, op1=OP.add)
                            onc = vtt.rearrange("p (hh c) -> p hh c",
                                                c=66)[:, :, 64:65]
                            nc.vector.tensor_copy(onc, ones_r.rearrange(
                                "p (hh c) -> p hh c", c=1))
                            vt_sb[i] = vtt
                    av = ps2.tile([65, HW], F32, tag="av", bufs=2,
                                  name=f"avh{h}")
                    for i in range(8):
                        eti = e_all.pop((h, i))
                        for n in range(2):
                            nc.tensor.matmul(
                                av[:, 512 * n:512 * (n + 1)],
                                vt_sb[i][:, 66 * h:66 * h + 65],
                                eti[:, 512 * n:512 * (n + 1)],
                                start=(i == 0), stop=(i == 7))

                    # per-head softmax division + residual
                    den_h = pool.tile([128, HW], F32, tag="denp", bufs=2,
                                      name=f"den{h}")
                    nc.vector.tensor_copy(den_h[64:65, :], av[64:65, :])
                    dzh = pool.tile([1, HW], F32, tag="dzh", bufs=2,
                                    name=f"dz{h}")
                    nc.sync.dma_start(dzh, den_h[64:65, :])
                    rzh = pool.tile([1, HW], F32, tag="rzh", bufs=2,
                                    name=f"rz{h}")
                    nc.vector.reciprocal_approx_fast(rzh, dzh)
                    rbs = pool.tile([64, HW], F32, tag="rb", bufs=2,
                                    name=f"rbs{h}")
                    nc.gpsimd.partition_broadcast(rbs, rzh)
                    tmp = pool.tile([64, HW], F32, tag="ftmp", bufs=2,
                                    name=f"tmp{h}")
                    nc.vector.tensor_tensor(out=tmp, in0=av[0:64, :],
                                            in1=rbs, op=OP.mult)
                    oh = pool.tile([64, HW], F32, tag="oh", bufs=2,
                                   name=f"oh{h}")
                    nc.vector.tensor_tensor(
                        out=oh, in0=tmp,
                        in1=xh4[:, HW * h:HW * (h + 1)], op=OP.add)
                    nc.sync.dma_start(
                        out_d.ap()[64 * h:64 * (h + 1), :], oh)

    nc.finalize()
    return nc


def _get_program():
    global _PROGRAM
    if _PROGRAM is None:
        _PROGRAM = _build_program()
    return _PROGRAM


def kernel(x, text_feat, gn_gamma, gn_beta, W0, b0, W1, b1, W2, b2):
    global _last_in_maps
    x = np.ascontiguousarray(np.asarray(x, dtype=np.float32))
    text_feat = np.ascontiguousarray(np.asarray(text_feat, dtype=np.float32))
    f32 = lambda a: np.ascontiguousarray(np.asarray(a, dtype=np.float32))
    W0, b0, W1, b1, W2, b2 = map(f32, (W0, b0, W1, b1, W2, b2))
    gn_gamma, gn_beta = f32(gn_gamma), f32(gn_beta)
    B = x.shape[0]

    gmat = np.zeros((CIN, NG), np.float32)
    for c in range(CIN):
        gmat[c, c // CPG] = 1.0 if c < C else float(HW)
    gmat_p = np.ascontiguousarray(
        gmat.reshape(6, 128, NG).transpose(1, 0, 2).reshape(128, 6 * NG))
    emat = np.zeros((NG, CIN), np.float32)
    for c in range(CIN):
        emat[c // CPG, c] = 1.0

    wall = np.empty((128, 1536), np.float32)
    for kc in range(2):
        for pi, W in enumerate((W0, W1, W2)):
            wall[:, 768 * kc + 256 * pi:768 * kc + 256 * (pi + 1)] = \
                W[:C][128 * kc:128 * (kc + 1), :]
    wtext = np.empty((128, 2048), np.float32)
    for kc in range(4):
        wtext[:, 512 * kc:512 * kc + 256] = W0[C:][128 * kc:128 * (kc + 1), :]
        wtext[:, 512 * kc + 256:512 * kc + 512] = \
            W2[C:][128 * kc:128 * (kc + 1), :]

    shared = {
        "gmat": gmat_p, "emat": emat, "wall": wall, "wtext": wtext,
        "b2row": b2.reshape(1, C),
    }
    in_maps = []
    for b in range(B):
        misc = np.zeros((128, 18), np.float32)
        misc[:, 0:4] = text_feat[b].reshape(4, 128).T
        misc[:, 4:10] = gn_gamma.reshape(6, 128).T
        misc[:, 10:16] = gn_beta.reshape(6, 128).T
        misc[:, 16:18] = b0.reshape(2, 128).T
        m = dict(shared)
        m["x"] = np.ascontiguousarray(x[b].reshape(C, HW))
        m["misc"] = misc
        in_maps.append(m)

    _last_in_maps = in_maps
    nc = _get_program()
    res = run_bass_kernel_spmd(nc, in_maps, core_ids=list(range(B)))
    out = np.stack([r["out"].reshape(C, 32, 32) for r in res.results])
    return out.astype(np.float32)



# revision 5
# speedup vs baseline: 1.0722x; 1.0722x over previous
"""AttnBlockWithText Trainium2 Bass kernel (v2: transposed-AV restructure).

Math (per batch element b, data-parallel over 8 NeuronCores):
  h   = concat([x_b, broadcast(text_b)])            # [768, 1024]
  hn  = GroupNorm(32, 768, eps=1e-6)(h) * gamma + beta
  q   = W0^T hn + b0 ; k = W1^T hn (bias dropped: softmax-invariant)
  v   = W2^T hn + b2; 4-head attention over 1024 positions; out = x + attn.

Key structure (on top of the analytic-text folding of the baseline):
  * Scores stay key-major: ss[kchunk 128, 1024 queries] = k_c^T q, f32r.
  * exp runs on ACT (bf16 out); selected stages run on DVE instead via a
    bf16 Schraudolph (i16 = A*s + B, bitcast bf16) to unload the ACT
    bottleneck (exp is the single largest engine cost: 32 x 1024 free).
  * AV is computed transposed: av[q 128, ch 64] += e_slice^T @ vT_chunk
    with the exp tile as the (cost-free) stationary operand and the small
    [128, 64] bf16 vT chunk as the moving operand -> 64 rows per matmul
    instead of 512.  The denominator comes from parallel free-1 matmuls
    (ones moving vector) sharing the same stationary.
  * With queries on partitions, the softmax division becomes a
    per-partition scalar: one fast-reciprocal + one fused
    (av * rinv + xT) scalar_tensor_tensor per (head, qchunk).
  * Output is produced as out_T [1024, 256] and transposed on the host.
"""

import sys

sys.path.insert(0, "/opt/trn_rl_repo")

import numpy as np

import concourse.bass as bass
import concourse.mybir as mybir
import concourse.tile as tile
from concourse import bacc
from concourse.bass_utils import run_bass_kernel_spmd

F32 = mybir.dt.float32
F32R = mybir.dt.float32r
BF16 = mybir.dt.bfloat16
I16 = mybir.dt.int16
AF = mybir.ActivationFunctionType
OP = mybir.AluOpType
AX = mybir.AxisListType

C = 256          # x channels
TC = 512         # text channels
CIN = C + TC     # 768
HW = 1024        # 32*32 spatial
NH = 4           # heads
NG = 32          # groupnorm groups
CPG = CIN // NG  # 24 channels per group
EPS = 1e-6
INV_CNT = 1.0 / (CPG * HW)

# Schraudolph bf16 exp: i16 = round(A * s + B); bitcast -> bf16 ~ exp(s/8).
# A = 0.125 * 128 / ln(2); B tuned below (includes +0.5 in case the store
# conversion truncates instead of rounds: s -> y is always positive).
EXP_A = 16.0 / np.log(2.0)
EXP_B = 16256.0 - 5.0

# Stage schedule: 32 stages (H, h, kcp).  Stages in this set compute exp on
# DVE (Schraudolph) instead of ACT.
EXP_DVE_STAGES = frozenset()

_PROGRAM = None
_last_in_maps = None


def _stages():
    out = []
    for Hh in range(2):
        for h in range(NH):
            for kcp in range(4):
                out.append((Hh, h, kcp))
    return out


def _build_program():
    nc = bacc.Bacc(None, target_bir_lowering=False)

    x_d = nc.dram_tensor("x", [C, HW], F32, kind="ExternalInput")
    xt_d = nc.dram_tensor("xt", [HW, C], F32, kind="ExternalInput")
    # packed small inputs: tcol[0:4] gam[4:10] bet[10:16] bias0[16:18]
    misc_d = nc.dram_tensor("misc", [128, 18], F32, kind="ExternalInput")
    b2r_d = nc.dram_tensor("b2row", [1, C], F32, kind="ExternalInput")
    gmat_d = nc.dram_tensor("gmat", [128, 6 * NG], F32, kind="ExternalInput")
    emat_d = nc.dram_tensor("emat", [NG, CIN], F32, kind="ExternalInput")
    # wall: [128, 2*3*256] f32r -- kc-major, then (W0,W1,W2)
    wall_d = nc.dram_tensor("wall", [128, 1536], F32R, kind="ExternalInput")
    # wtext: [128, 4*(256+256)] f32 -- kc-major, then (W0t, W2t)
    wtext_d = nc.dram_tensor("wtext", [128, 2048], F32, kind="ExternalInput")
    out_d = nc.dram_tensor("out", [HW, C], F32, kind="ExternalOutput")

    with tile.TileContext(nc) as tc:
        with tc.tile_pool(name="sb", bufs=1) as pool:
            # ---------------- persistent inputs ----------------
            x_sb = []
            for m in range(2):
                xt_ = pool.tile([128, HW], F32, name=f"x{m}")
                for p in range(2):
                    nc.sync.dma_start(
                        xt_[64 * p:64 * (p + 1), :],
                        x_d.ap()[128 * m + 64 * p:128 * m + 64 * (p + 1), :])
                x_sb.append(xt_)
            misc = pool.tile([128, 18], F32, name="misc_sb")
            nc.sync.dma_start(misc, misc_d.ap())
            gm = pool.tile([128, 6 * NG], F32, name="gm_sb")
            nc.sync.dma_start(gm, gmat_d.ap())
            em = pool.tile([NG, CIN], F32, name="em_sb")
            nc.sync.dma_start(em, emat_d.ap())
            # weights: W0 cols first (q), then W1 (k), then text, then W2 (v)
            wall = pool.tile([128, 1536], F32R, name="wall_sb")
            for pi in range(3):
                for kc in range(2):
                    sl = slice(768 * kc + 256 * pi, 768 * kc + 256 * (pi + 1))
                    nc.sync.dma_start(wall[:, sl], wall_d.ap()[:, sl])
            wtext = pool.tile([128, 2048], F32, name="wtext_sb")
            nc.sync.dma_start(wtext, wtext_d.ap())
            b2r = pool.tile([1, C], F32, name="b2r_sb")
            nc.sync.dma_start(b2r, b2r_d.ap())
            # transposed x for the residual add (needed late)
            xT = pool.tile([128, 8 * C], F32, name="xT")
            nc.sync.dma_start(
                xT.rearrange("p (j c) -> p j c", c=C),
                xt_d.ap().rearrange("(j p) c -> p j c", p=128))

            tcol = misc[:, 0:4]
            gam6 = misc[:, 4:10]
            bet6 = misc[:, 10:16]
            bias0 = misc[:, 16:18]
            wq = [wall[:, 768 * kc + 0:768 * kc + 256] for kc in range(2)]
            wk = [wall[:, 768 * kc + 256:768 * kc + 512] for kc in range(2)]
            wv = [wall[:, 768 * kc + 512:768 * kc + 768] for kc in range(2)]
            w0t = [wtext[:, 512 * kc:512 * kc + 256] for kc in range(4)]
            w2t = [wtext[:, 512 * kc + 256:512 * kc + 512] for kc in range(4)]

            ones_bf = pool.tile([128, 1], BF16, name="ones_bf")
            nc.vector.memset(ones_bf, 1.0)

            with tc.tile_pool(name="ps1", bufs=1, space="PSUM") as ps1:
                # ---------------- group statistics ----------------
                st = []
                for cc in range(2):
                    stt = pool.tile([128, 2], F32, name=f"st{cc}")
                    scratch = pool.tile([128, HW], F32, tag="scr", bufs=2,
                                        name=f"scr{cc}")
                    # sum(x^2) on ScalarE, sum(x) on DVE; per partition-half
                    # so each starts as its DMA lands
                    for p in range(2):
                        sl = slice(64 * p, 64 * (p + 1))
                        nc.scalar.activation(scratch[sl, :], x_sb[cc][sl, :],
                                             AF.Square,
                                             accum_out=stt[sl, 1:2])
                        nc.vector.reduce_sum(stt[sl, 0:1], x_sb[cc][sl, :],
                                             axis=AX.X)
                    st.append(stt)
                for j in range(4):
                    stt = pool.tile([128, 2], F32, name=f"stt{j}")
                    nc.vector.tensor_copy(stt[:, 0:1], tcol[:, j:j + 1])
                    nc.vector.tensor_scalar(
                        out=stt[:, 1:2], in0=tcol[:, j:j + 1],
                        scalar1=tcol[:, j:j + 1], scalar2=None, op0=OP.mult)
                    st.append(stt)

                ps_st = ps1.tile([NG, 2], F32, tag="sps", bufs=2,
                                 name="ps_st")
                for cc in range(6):
                    nc.tensor.matmul(ps_st, gm[:, NG * cc:NG * (cc + 1)],
                                     st[cc], start=(cc == 0), stop=(cc == 5))

                sms = pool.tile([NG, 2], F32, name="sms")
                nc.vector.tensor_scalar(out=sms, in0=ps_st, scalar1=INV_CNT,
                                        scalar2=None, op0=OP.mult)
                mu = sms[:, 0:1]
                m2 = sms[:, 1:2]
                nvar = pool.tile([NG, 1], F32, name="nvar")
                nc.vector.scalar_tensor_tensor(out=nvar, in0=mu, scalar=mu,
                                               in1=m2, op0=OP.mult,
                                               op1=OP.subtract)
                veps = pool.tile([NG, 1], F32, name="veps")
                nc.vector.tensor_scalar(out=veps, in0=nvar, scalar1=-1.0,
                                        scalar2=EPS, op0=OP.mult, op1=OP.add)
                # rsqrt: linear seed + 3 Newton steps (var ~ 1 here)
                ya = pool.tile([NG, 1], F32, name="ya")
                yb = pool.tile([NG, 1], F32, name="yb")
                t2 = pool.tile([NG, 1], F32, name="t2c")
                uu = pool.tile([NG, 1], F32, name="uu")
                nc.vector.tensor_scalar(out=ya, in0=veps, scalar1=-0.5,
                                        scalar2=1.5, op0=OP.mult, op1=OP.add)
                cur, nxt = ya, yb
                for it in range(3):
                    nc.vector.tensor_scalar(out=t2, in0=veps, scalar1=cur,
                                            scalar2=cur, op0=OP.mult,
                                            op1=OP.mult)
                    nc.vector.tensor_scalar(out=uu, in0=t2, scalar1=-0.5,
                                            scalar2=1.5, op0=OP.mult,
                                            op1=OP.add)
                    dst = sms[:, 1:2] if it == 2 else nxt
                    nc.vector.tensor_scalar(out=dst, in0=cur, scalar1=uu,
                                            scalar2=None, op0=OP.mult)
                    cur, nxt = nxt, cur
                mr = sms

                # expand per-group (mu, rsqrt) to per-channel
                pse = ps1.tile([128, 12], F32, tag="sps", bufs=2, name="pse")
                for cc in range(6):
                    nc.tensor.matmul(pse[:, 2 * cc:2 * (cc + 1)],
                                     em[:, 128 * cc:128 * (cc + 1)],
                                     mr, start=True, stop=True)
                pse_mu = pse.rearrange("p (c two) -> p c two", two=2)[:, :, 0]
                pse_rs = pse.rearrange("p (c two) -> p c two", two=2)[:, :, 1]
                sc6 = pool.tile([128, 6], F32, name="sc6")
                nc.vector.tensor_tensor(out=sc6, in0=pse_rs, in1=gam6,
                                        op=OP.mult)
                mg6 = pool.tile([128, 6], F32, name="mg6")
                nc.vector.tensor_tensor(out=mg6, in0=pse_mu, in1=sc6,
                                        op=OP.mult)
                ngt6 = pool.tile([128, 6], F32, name="ngt6")
                nc.vector.tensor_tensor(out=ngt6, in0=mg6, in1=bet6,
                                        op=OP.subtract)  # = mu*s - beta
                # negated shift for the ACT-side hn (out = s*x + (-(mu*s-b)))
                ngt6n = pool.tile([128, 6], F32, name="ngt6n")
                nc.vector.tensor_scalar(out=ngt6n, in0=ngt6, scalar1=-1.0,
                                        scalar2=None, op0=OP.mult)

                # normalized x channels; chunk 0 on ACT, chunk 1 on DVE
                hn = []
                for cc in range(2):
                    hnt = pool.tile([128, HW], F32R, name=f"hn{cc}")
                    if cc == 0:
                        nc.scalar.activation(hnt, x_sb[cc], AF.Identity,
                                             bias=ngt6n[:, cc:cc + 1],
                                             scale=sc6[:, cc:cc + 1])
                    else:
                        nc.vector.tensor_scalar(out=hnt, in0=x_sb[cc],
                                                scalar1=sc6[:, cc:cc + 1],
                                                scalar2=ngt6[:, cc:cc + 1],
                                                op0=OP.mult, op1=OP.subtract)
                    hn.append(hnt)
                # normalized text channels (constant over space): [128,1] x4
                hnt_cols = []
                for j in range(4):
                    ht = pool.tile([128, 1], F32, name=f"hnt{j}")
                    nc.vector.tensor_scalar(out=ht, in0=tcol[:, j:j + 1],
                                            scalar1=sc6[:, 2 + j:3 + j],
                                            scalar2=ngt6[:, 2 + j:3 + j],
                                            op0=OP.mult, op1=OP.subtract)
                    hnt_cols.append(ht)

                # q bias = W0t^T hn_t + b0 (per-channel col), v text row
                qb_cols = []
                for m in range(2):
                    psq = ps1.tile([128, 1], F32, tag="sps", bufs=2,
                                   name=f"psqb{m}")
                    for kc in range(4):
                        nc.tensor.matmul(
                            psq, w0t[kc][:, 128 * m:128 * (m + 1)],
                            hnt_cols[kc], start=(kc == 0), stop=(kc == 3))
                    qb = pool.tile([128, 1], F32, name=f"qb{m}")
                    nc.vector.tensor_scalar(out=qb, in0=psq,
                                            scalar1=bias0[:, m:m + 1],
                                            scalar2=None, op0=OP.add)
                    qb_cols.append(qb)
                ps_vtx = ps1.tile([1, C], F32, tag="sps", bufs=2,
                                  name="ps_vtx")
                for kc in range(4):
                    nc.tensor.matmul(ps_vtx, hnt_cols[kc], w2t[kc],
                                     start=(kc == 0), stop=(kc == 3))
                vtext = pool.tile([1, C], F32, name="vtext")
                nc.vector.tensor_tensor(out=vtext, in0=ps_vtx, in1=b2r,
                                        op=OP.add)
                # broadcast vtext over all partitions for the vT epilogue
                vtext_b = pool.tile([128, C], F32, name="vtext_b")
                nc.gpsimd.partition_broadcast(vtext_b, vtext)

                # ---------------- q, k projections ----------------
                # m=0 (heads 0,1) first and fully finished before m=1 so the
                # first score stage starts as early as possible.
                q_sb = [pool.tile([128, HW], F32R, name=f"q{m}")
                        for m in range(2)]
                k_sb = [pool.tile([128, HW], F32R, name=f"k{m}")
                        for m in range(2)]
                for m in range(2):
                    psqm = ps1.tile([128, HW], F32, tag="qk", bufs=2,
                                    name=f"psq{m}")
                    for kc in range(2):
                        for n in range(2):
                            nc.tensor.matmul(
                                psqm[:, 512 * n:512 * (n + 1)],
                                wq[kc][:, 128 * m:128 * (m + 1)],
                                hn[kc][:, 512 * n:512 * (n + 1)],
                                start=(kc == 0), stop=(kc == 1))
                    pskm = ps1.tile([128, HW], F32, tag="qk", bufs=2,
                                    name=f"psk{m}")
                    for kc in range(2):
                        for n in range(2):
                            nc.tensor.matmul(
                                pskm[:, 512 * n:512 * (n + 1)],
                                wk[kc][:, 128 * m:128 * (m + 1)],
                                hn[kc][:, 512 * n:512 * (n + 1)],
                                start=(kc == 0), stop=(kc == 1))
                    # q bias-add + copy to SBUF: ACT for m=0 (the critical
                    # one), DVE for m=1 (keeps the ACT exp stream clean)
                    if m == 0:
                        nc.scalar.activation(q_sb[m], psqm, AF.Identity,
                                             bias=qb_cols[m], scale=1.0)
                    else:
                        nc.vector.tensor_scalar(out=q_sb[m], in0=psqm,
                                                scalar1=qb_cols[m],
                                                scalar2=None, op0=OP.add)
                    # k copy: first 256 cols first (stage 0 needs kc 0,1)
                    nc.vector.tensor_copy(k_sb[m][:, 0:256],
                                          pskm[:, 0:256])
                    nc.vector.tensor_copy(k_sb[m][:, 256:1024],
                                          pskm[:, 256:1024])

            # ---------------- attention ----------------
            stages = _stages()
            with tc.tile_pool(name="ps2", bufs=1, space="PSUM") as ps2:
                # den: one persistent bank; col = 16H + 4j + h
                den = ps2.tile([128, 32], F32, name="den")
                av = {}       # (H, pair) -> [128, 512] psum tile
                e_all = {}    # stage -> e tile
                vtt = [None] * 8
                ostage = [pool.tile([128, HW], F32, name=f"ost{Hh}")
                          for Hh in range(2)]

                def emit_v_chunk(i):
                    psv = ps2.tile([128, C], F32, tag="pv", bufs=1,
                                   name=f"psv{i}")
                    for kc in range(2):
                        nc.tensor.matmul(
                            psv, hn[kc][:, 128 * i:128 * (i + 1)],
                            wv[kc], start=(kc == 0), stop=(kc == 1))
                    vt = pool.tile([128, C], BF16, name=f"vt{i}")
                    vt4 = vt.rearrange("p (hh c) -> p hh c", c=64)
                    nc.vector.scalar_tensor_tensor(
                        out=vt4, in0=psv.rearrange("p (hh c) -> p hh c",
                                                   c=64),
                        scalar=1.0,
                        in1=vtext_b.rearrange("p (hh c) -> p hh c", c=64),
                        op0=OP.bypass, op1=OP.add)
                    vtt[i] = vt

                def emit_scores(s):
                    Hh, h, kcp = stages[s]
                    m, r = h // 2, h % 2
                    ss = ps2.tile([128, HW], F32, tag="sc", bufs=2,
                                  name=f"ss{s}")
                    for t in range(2):
                        kc = 2 * kcp + t
                        nc.tensor.matmul(
                            ss[:, 512 * t:512 * (t + 1)],
                            k_sb[m][64 * r:64 * (r + 1),
                                    128 * kc:128 * (kc + 1)],
                            q_sb[m][64 * r:64 * (r + 1),
                                    512 * Hh:512 * (Hh + 1)],
                            start=True, stop=True,
                            tile_position=(64 * r, 0))
                    return ss

                def emit_exp(s, ss):
                    et = pool.tile([128, HW], BF16, tag="e", bufs=4,
                                   name=f"e{s}")
                    if s in EXP_DVE_STAGES:
                        nc.vector.tensor_scalar(
                            out=et.bitcast(I16), in0=ss,
                            scalar1=float(EXP_A), scalar2=float(EXP_B),
                            op0=OP.mult, op1=OP.add)
                    else:
                        nc.scalar.activation(et, ss, AF.Exp, scale=0.125)
                    e_all[s] = et

                def emit_av(s):
                    Hh, h, kcp = stages[s]
                    et = e_all.pop(s)
                    if (Hh, 0) not in av:
                        for pr in range(2):
                            av[(Hh, pr)] = ps2.tile(
                                [128, 512], F32, tag="av", bufs=2,
                                name=f"av{Hh}{pr}")
                    first = kcp == 0
                    last = kcp == 3
                    for t in range(2):
                        kc = 2 * kcp + t
                        for j in range(4):
                            lhs = et[:, 512 * t + 128 * j:
                                     512 * t + 128 * (j + 1)]
                            nc.tensor.matmul(
                                av[(Hh, j // 2)][:, 256 * (j % 2) + 64 * h:
                                                 256 * (j % 2) + 64 * (h + 1)],
                                lhs, vtt[kc][:, 64 * h:64 * (h + 1)],
                                start=(first and t == 0),
                                stop=(last and t == 1))
                            nc.tensor.matmul(
                                den[:, 16 * Hh + 4 * j + h:
                                    16 * Hh + 4 * j + h + 1],
                                lhs, ones_bf,
                                start=(first and t == 0),
                                stop=(last and t == 1))

                def emit_head_tail(s):
                    Hh, h, _ = stages[s]
                    # rinv[j] = 1 / den[:, 16H + 4j + h] for j = 0..3
                    rinv = pool.tile([128, 4], F32, tag="ri", bufs=4,
                                     name=f"ri{Hh}{h}")
                    dsl = den.rearrange("p (g j h) -> p g j h",
                                        g=2, j=4, h=4)[:, Hh, :, h]
                    nc.vector.reciprocal_approx_fast(rinv, dsl)
                    for j in range(4):
                        nc.vector.scalar_tensor_tensor(
                            out=ostage[Hh][:, 256 * j + 64 * h:
                                           256 * j + 64 * (h + 1)],
                            in0=av[(Hh, j // 2)][:, 256 * (j % 2) + 64 * h:
                                                 256 * (j % 2) + 64 * (h + 1)],
                            scalar=rinv[:, j:j + 1],
                            in1=xT[:, 256 * (4 * Hh + j) + 64 * h:
                                   256 * (4 * Hh + j) + 64 * (h + 1)],
                            op0=OP.mult, op1=OP.add)
                    if h == 3:
                        for pr in range(2):
                            del av[(Hh, pr)]
                        for j in range(4):
                            qi = 4 * Hh + j
                            nc.sync.dma_start(
                                out_d.ap()[128 * qi:128 * (qi + 1), :],
                                ostage[Hh][:, 256 * j:256 * (j + 1)])

                for s in range(len(stages)):
                    ss = emit_scores(s)
                    # spread the 8 v-chunk builds across the first 4 stages
                    if s < 4:
                        emit_v_chunk(2 * s)
                        emit_v_chunk(2 * s + 1)
                    emit_exp(s, ss)
                    if s >= 1:
                        emit_av(s - 1)
                        if stages[s - 1][2] == 3:
                            emit_head_tail(s - 1)
                emit_av(31)
                emit_head_tail(31)

    nc.finalize()
    return nc


def _get_program():
    global _PROGRAM
    if _PROGRAM is None:
        _PROGRAM = _build_program()
    return _PROGRAM


def kernel(x, text_feat, gn_gamma, gn_beta, W0, b0, W1, b1, W2, b2):
    global _last_in_maps
    x = np.ascontiguousarray(np.asarray(x, dtype=np.float32))
    text_feat = np.ascontiguousarray(np.asarray(text_feat, dtype=np.float32))
    f32 = lambda a: np.ascontiguousarray(np.asarray(a, dtype=np.float32))
    W0, b0, W1, b1, W2, b2 = map(f32, (W0, b0, W1, b1, W2, b2))
    gn_gamma, gn_beta = f32(gn_gamma), f32(gn_beta)
    B = x.shape[0]

    gmat = np.zeros((CIN, NG), np.float32)
    for c in range(CIN):
        gmat[c, c // CPG] = 1.0 if c < C else float(HW)
    gmat_p = np.ascontiguousarray(
        gmat.reshape(6, 128, NG).transpose(1, 0, 2).reshape(128, 6 * NG))
    emat = np.zeros((NG, CIN), np.float32)
    for c in range(CIN):
        emat[c // CPG, c] = 1.0

    wall = np.empty((128, 1536), np.float32)
    for kc in range(2):
        for pi, W in enumerate((W0, W1, W2)):
            wall[:, 768 * kc + 256 * pi:768 * kc + 256 * (pi + 1)] = \
                W[:C][128 * kc:128 * (kc + 1), :]
    wtext = np.empty((128, 2048), np.float32)
    for kc in range(4):
        wtext[:, 512 * kc:512 * kc + 256] = W0[C:][128 * kc:128 * (kc + 1), :]
        wtext[:, 512 * kc + 256:512 * kc + 512] = \
            W2[C:][128 * kc:128 * (kc + 1), :]

    shared = {
        "gmat": gmat_p, "emat": emat, "wall": wall, "wtext": wtext,
        "b2row": b2.reshape(1, C),
    }
    in_maps = []
    for b in range(B):
        misc = np.zeros((128, 18), np.float32)
        misc[:, 0:4] = text_feat[b].reshape(4, 128).T
        misc[:, 4:10] = gn_gamma.reshape(6, 128).T
        misc[:, 10:16] = gn_beta.reshape(6, 128).T
        misc[:, 16:18] = b0.reshape(2, 128).T
        m = dict(shared)
        xb = x[b].reshape(C, HW)
        m["x"] = np.ascontiguousarray(xb)
        m["xt"] = np.ascontiguousarray(xb.T)
        m["misc"] = misc
        in_maps.append(m)

    _last_in_maps = in_maps
    nc = _get_program()
    res = run_bass_kernel_spmd(nc, in_maps, core_ids=list(range(B)))
    out = np.stack([r["out"].reshape(HW, C).T.reshape(C, 32, 32)
                    for r in res.results])
    return np.ascontiguousarray(out).astype(np.float32)


# revision 7
# speedup vs baseline: 1.0856x; 1.0125x over previous
"""AttnBlockWithText Trainium2 Bass kernel (v3).

Math (per batch element b, data-parallel over 8 NeuronCores):
  h   = concat([x_b, broadcast(text_b)])            # [768, 1024]
  hn  = GroupNorm(32, 768, eps=1e-6)(h) * gamma + beta
  q   = W0^T hn + b0 ; k = W1^T hn (k bias dropped: softmax-invariant)
  v   = W2^T hn + b2; 4-head attention over 1024 positions; out = x + attn.

Structure (on top of the analytic text-channel folding):
  * GroupNorm is folded into the weights: W' = s (.) W computed on-chip with
    tiny per-partition tensor_scalar ops, so projections consume raw x and
    the mean-shift becomes extra bias columns (computed with free-1 PE
    matmuls).  No normalized-activation tiles are ever materialized.
  * rsqrt(var) = recip_fast(ACT sqrt) instead of a Newton ladder.
  * Scores stay key-major: ss[kchunk 128, 1024 queries] = k_c^T q in f32r.
  * exp on ACT (bf16 out); selected stages instead use a bf16 Schraudolph
    on DVE (i16 = A*s + B bitcast bf16, ~2% err) to unload the ACT
    bottleneck (exp is the largest single engine cost: 32 x 1024 free).
  * AV computed transposed: av[q 128, ch 64] += e_slice^T @ vT_chunk with
    the exp tile as (cost-free) stationary and the small [128, 64] bf16 vT
    chunk moving -> 64 rows per matmul instead of 512.  Denominators come
    from parallel free-1 matmuls (ones vector) sharing the stationary.
  * With queries on partitions the softmax division is a per-partition
    scalar: fast-reciprocal + one fused (av * rinv + xT) op per
    (head, qchunk).  Output is out_T [1024, 256], transposed on the host.
  * One PSUM pool for the whole program; the score ring reuses the
    projection banks tile-by-tile (no pool-transition barrier).
"""

import sys

sys.path.insert(0, "/opt/trn_rl_repo")

import numpy as np

import concourse.bass as bass
import concourse.mybir as mybir
import concourse.tile as tile
from concourse import bacc
from concourse.bass_utils import run_bass_kernel_spmd

F32 = mybir.dt.float32
F32R = mybir.dt.float32r
BF16 = mybir.dt.bfloat16
I16 = mybir.dt.int16
AF = mybir.ActivationFunctionType
OP = mybir.AluOpType
AX = mybir.AxisListType

C = 256          # x channels
TC = 512         # text channels
CIN = C + TC     # 768
HW = 1024        # 32*32 spatial
NH = 4           # heads
NG = 32          # groupnorm groups
CPG = CIN // NG  # 24 channels per group
EPS = 1e-6
INV_CNT = 1.0 / (CPG * HW)

# Schraudolph bf16 exp for DVE-offloaded stages:
#   i16 = round(s * A + B); bitcast bf16 ~= exp(0.125 * s).
EXP_A = 16.0 / np.log(2.0)
EXP_B = 16251.0

# Stages (index 0..31) whose exp runs on DVE instead of ACT.
EXP_DVE_STAGES = frozenset()

_PROGRAM = None
_last_in_maps = None


def _stages():
    out = []
    for Hh in range(2):
        for h in range(NH):
            for kcp in range(4):
                out.append((Hh, h, kcp))
    return out


def _build_program():
    nc = bacc.Bacc(None, target_bir_lowering=False)

    x_d = nc.dram_tensor("x", [C, HW], F32, kind="ExternalInput")
    xt_d = nc.dram_tensor("xt", [HW, C], F32, kind="ExternalInput")
    # packed small inputs: tcol[0:4] gam[4:10] bet[10:16] bias0[16:18]
    misc_d = nc.dram_tensor("misc", [128, 18], F32, kind="ExternalInput")
    b2r_d = nc.dram_tensor("b2row", [1, C], F32, kind="ExternalInput")
    gmat_d = nc.dram_tensor("gmat", [128, 6 * NG], F32, kind="ExternalInput")
    emat_d = nc.dram_tensor("emat", [NG, CIN], F32, kind="ExternalInput")
    # wall: [128, 2*3*256] f32r -- kc-major, then (W0,W1,W2)
    wall_d = nc.dram_tensor("wall", [128, 1536], F32R, kind="ExternalInput")
    # wtext: [128, 4*(256+256)] bf16 -- kc-major, then (W0t, W2t)
    wtext_d = nc.dram_tensor("wtext", [128, 2048], BF16, kind="ExternalInput")
    out_d = nc.dram_tensor("out", [HW, C], F32, kind="ExternalOutput")

    stages = _stages()

    with tile.TileContext(nc) as tc:
        with tc.tile_pool(name="sb", bufs=1) as pool, \
             tc.tile_pool(name="ps", bufs=1, space="PSUM") as ps:
            # ---------------- input DMAs (issue order = priority) --------
            x_sb = []
            for m in range(2):
                xm = pool.tile([128, HW], F32, name=f"x{m}")
                for p in range(2):
                    nc.sync.dma_start(
                        xm[64 * p:64 * (p + 1), :],
                        x_d.ap()[128 * m + 64 * p:128 * m + 64 * (p + 1), :])
                x_sb.append(xm)
            misc = pool.tile([128, 18], F32, name="misc_sb")
            nc.sync.dma_start(misc, misc_d.ap())
            gm = pool.tile([128, 6 * NG], F32, name="gm_sb")
            nc.sync.dma_start(gm, gmat_d.ap())
            em = pool.tile([NG, CIN], F32, name="em_sb")
            nc.sync.dma_start(em, emat_d.ap())
            wtext = pool.tile([128, 2048], BF16, name="wtext_sb")
            nc.sync.dma_start(wtext, wtext_d.ap())
            wall = pool.tile([128, 1536], F32R, name="wall_sb")
            for pi in range(3):          # W0 first (q), then W1, then W2
                for kc in range(2):
                    sl = slice(768 * kc + 256 * pi, 768 * kc + 256 * (pi + 1))
                    nc.sync.dma_start(wall[:, sl], wall_d.ap()[:, sl])
            b2r = pool.tile([1, C], F32, name="b2r_sb")
            nc.sync.dma_start(b2r, b2r_d.ap())
            xT = pool.tile([128, 8 * C], F32, name="xT")
            nc.sync.dma_start(
                xT.rearrange("p (j c) -> p j c", c=C),
                xt_d.ap().rearrange("(j p) c -> p j c", p=128))

            tcol = misc[:, 0:4]
            gam6 = misc[:, 4:10]
            bet6 = misc[:, 10:16]
            bias0 = misc[:, 16:18]
            wq = [wall[:, 768 * kc + 0:768 * kc + 256] for kc in range(2)]
            wk = [wall[:, 768 * kc + 256:768 * kc + 512] for kc in range(2)]
            wv = [wall[:, 768 * kc + 512:768 * kc + 768] for kc in range(2)]
            w0t = [wtext[:, 512 * kc:512 * kc + 256] for kc in range(4)]
            w2t = [wtext[:, 512 * kc + 256:512 * kc + 512] for kc in range(4)]

            ones_bf = pool.tile([128, 1], BF16, name="ones_bf")
            nc.vector.memset(ones_bf, 1.0)

            # ---------------- group statistics ----------------
            st = []
            for cc in range(2):
                stt = pool.tile([128, 2], F32, name=f"st{cc}")
                scratch = pool.tile([128, HW], F32, tag="scr", bufs=2,
                                    name=f"scr{cc}")
                for p in range(2):
                    sl = slice(64 * p, 64 * (p + 1))
                    nc.scalar.activation(scratch[sl, :], x_sb[cc][sl, :],
                                         AF.Square, accum_out=stt[sl, 1:2])
                    nc.vector.reduce_sum(stt[sl, 0:1], x_sb[cc][sl, :],
                                         axis=AX.X)
                st.append(stt)
            for j in range(4):
                stt = pool.tile([128, 2], F32, name=f"stt{j}")
                nc.vector.tensor_copy(stt[:, 0:1], tcol[:, j:j + 1])
                nc.vector.tensor_scalar(
                    out=stt[:, 1:2], in0=tcol[:, j:j + 1],
                    scalar1=tcol[:, j:j + 1], scalar2=None, op0=OP.mult)
                st.append(stt)

            ps_st = ps.tile([NG, 2], F32, tag="sps", bufs=1, name="ps_st")
            for cc in range(6):
                nc.tensor.matmul(ps_st, gm[:, NG * cc:NG * (cc + 1)],
                                 st[cc], start=(cc == 0), stop=(cc == 5))

            # mean, rsqrt(var + eps): sqrt on ACT + fast reciprocal on DVE
            sms = pool.tile([NG, 2], F32, name="sms")
            nc.vector.tensor_scalar(out=sms, in0=ps_st, scalar1=INV_CNT,
                                    scalar2=None, op0=OP.mult)
            mu = sms[:, 0:1]
            veps = pool.tile([NG, 2], F32, name="veps")
            # veps = m2 - mu*mu + eps  ==  (mu * -mu) + m2, then + eps
            nc.vector.scalar_tensor_tensor(out=veps[:, 0:1], in0=mu,
                                           scalar=mu, in1=sms[:, 1:2],
                                           op0=OP.mult, op1=OP.subtract)
            nc.vector.tensor_scalar(out=veps[:, 1:2], in0=veps[:, 0:1],
                                    scalar1=-1.0, scalar2=EPS,
                                    op0=OP.mult, op1=OP.add)
            sdev = pool.tile([NG, 1], F32, name="sdev")
            nc.scalar.activation(sdev, veps[:, 1:2], AF.Sqrt)
            nc.vector.reciprocal_approx_fast(sms[:, 1:2], sdev)
            mr = sms

            # expand per-group (mu, rsqrt) to per-channel [128, 6x2]
            pse = ps.tile([128, 12], F32, tag="sps", bufs=1, name="pse")
            for cc in range(6):
                nc.tensor.matmul(pse[:, 2 * cc:2 * (cc + 1)],
                                 em[:, 128 * cc:128 * (cc + 1)],
                                 mr, start=True, stop=True)
            pse_mu = pse.rearrange("p (c two) -> p c two", two=2)[:, :, 0]
            pse_rs = pse.rearrange("p (c two) -> p c two", two=2)[:, :, 1]
            sc6 = pool.tile([128, 6], F32, name="sc6")
            nc.vector.tensor_tensor(out=sc6, in0=pse_rs, in1=gam6, op=OP.mult)
            mg6 = pool.tile([128, 6], F32, name="mg6")
            nc.vector.tensor_tensor(out=mg6, in0=pse_mu, in1=sc6, op=OP.mult)
            # tneg = beta - mu*s  (f32r for use as a matmul operand)
            tneg = pool.tile([128, 6], F32R, name="tneg")
            nc.vector.tensor_tensor(out=tneg, in0=bet6, in1=mg6,
                                    op=OP.subtract)

            # normalized text channels (constant over space), bf16 cols
            hnt_cols = []
            for j in range(4):
                ht = pool.tile([128, 1], BF16, name=f"hnt{j}")
                nc.vector.scalar_tensor_tensor(
                    out=ht, in0=tcol[:, j:j + 1],
                    scalar=sc6[:, 2 + j:3 + j],
                    in1=tneg[:, 2 + j:3 + j].bitcast(F32),
                    op0=OP.mult, op1=OP.add)
                hnt_cols.append(ht)

            # folded weights W' = s (.) W  (q on DVE: critical path;
            # k, v on GPSIMD to keep DVE free)
            wqp, wkp, wvp = [], [], []
            for kc in range(2):
                t = pool.tile([128, 256], F32R, name=f"wqp{kc}")
                nc.vector.tensor_scalar(out=t, in0=wq[kc],
                                        scalar1=sc6[:, kc:kc + 1],
                                        scalar2=None, op0=OP.mult)
                wqp.append(t)
            for kc in range(2):
                t = pool.tile([128, 256], F32R, name=f"wkp{kc}")
                nc.gpsimd.tensor_scalar(out=t, in0=wk[kc],
                                        scalar1=sc6[:, kc:kc + 1],
                                        scalar2=None, op0=OP.mult)
                wkp.append(t)
            for kc in range(2):
                t = pool.tile([128, 256], F32R, name=f"wvp{kc}")
                nc.gpsimd.tensor_scalar(out=t, in0=wv[kc],
                                        scalar1=sc6[:, kc:kc + 1],
                                        scalar2=None, op0=OP.mult)
                wvp.append(t)

            # q bias cols: b0 + W0t^T hn_text + W0x^T tneg   [128,1] per m
            qb_cols = []
            for m in range(2):
                psq = ps.tile([128, 1], F32, tag="sps", bufs=1,
                              name=f"psqb{m}")
                for kc in range(4):
                    nc.tensor.matmul(
                        psq, w0t[kc][:, 128 * m:128 * (m + 1)],
                        hnt_cols[kc], start=(kc == 0), stop=False)
                for kc in range(2):
                    nc.tensor.matmul(
                        psq, wq[kc][:, 128 * m:128 * (m + 1)],
                        tneg[:, kc:kc + 1], start=False, stop=(kc == 1))
                qb = pool.tile([128, 1], F32, name=f"qb{m}")
                nc.vector.tensor_scalar(out=qb, in0=psq,
                                        scalar1=bias0[:, m:m + 1],
                                        scalar2=None, op0=OP.add)
                qb_cols.append(qb)

            # v const row: b2 + W2t^T hn_text + W2x^T tneg   [1, C]
            ps_vtx = ps.tile([1, C], F32, tag="sps", bufs=1, name="ps_vtx")
            for kc in range(4):
                nc.tensor.matmul(ps_vtx, hnt_cols[kc], w2t[kc],
                                 start=(kc == 0), stop=False)
            for kc in range(2):
                nc.tensor.matmul(ps_vtx, tneg[:, kc:kc + 1], wv[kc],
                                 start=False, stop=(kc == 1))
            vtext = pool.tile([1, C], F32, name="vtext")
            nc.vector.tensor_tensor(out=vtext, in0=ps_vtx, in1=b2r,
                                    op=OP.add)
            vtext_b = pool.tile([128, C], F32, name="vtext_b")
            nc.gpsimd.partition_broadcast(vtext_b, vtext)

            # ---------------- m=0 projections (critical path) -----------
            xr = [x_sb[kc].bitcast(F32R) for kc in range(2)]
            q_sb = [pool.tile([128, HW], F32R, name=f"q{m}")
                    for m in range(2)]
            k_sb = [pool.tile([128, HW], F32R, name=f"k{m}")
                    for m in range(2)]

            psq0 = ps.tile([128, HW], F32, tag="qk", bufs=2, name="psq0")
            for n in range(2):
                for kc in range(2):
                    nc.tensor.matmul(psq0[:, 512 * n:512 * (n + 1)],
                                     wqp[kc][:, 0:128],
                                     xr[kc][:, 512 * n:512 * (n + 1)],
                                     start=(kc == 0), stop=(kc == 1))
            # bias-add + copy: first query half on ACT, second on DVE
            nc.scalar.activation(q_sb[0][:, 0:512], psq0[:, 0:512],
                                 AF.Identity, bias=qb_cols[0], scale=1.0)
            nc.vector.tensor_scalar(out=q_sb[0][:, 512:1024],
                                    in0=psq0[:, 512:1024],
                                    scalar1=qb_cols[0], scalar2=None,
                                    op0=OP.add)
            psk0 = ps.tile([128, HW], F32, tag="qk", bufs=2, name="psk0")
            for n in range(2):
                for kc in range(2):
                    nc.tensor.matmul(psk0[:, 512 * n:512 * (n + 1)],
                                     wkp[kc][:, 0:128],
                                     xr[kc][:, 512 * n:512 * (n + 1)],
                                     start=(kc == 0), stop=(kc == 1))
            # k copy in pieces so stage 0 (cols 0:256) unblocks early
            nc.vector.tensor_copy(k_sb[0][:, 0:256], psk0[:, 0:256])
            nc.vector.tensor_copy(k_sb[0][:, 256:512], psk0[:, 256:512])
            nc.vector.tensor_copy(k_sb[0][:, 512:1024], psk0[:, 512:1024])

            # ---------------- attention ----------------
            # den: persistent column bank; col = 16H + 4j + h
            den = ps.tile([128, 32], F32, tag="sps", bufs=1, name="den")
            av = {}
            e_all = {}
            vtt = [None] * 8
            ostage = [pool.tile([128, HW], F32, name=f"ost{Hh}")
                      for Hh in range(2)]

            # m=1 projections through a single spare bank, in halves;
            # emitted interleaved with the first stages (needed at stage 8)
            def emit_m1_piece(i):
                half = i % 2
                csl = slice(512 * half, 512 * (half + 1))
                if i < 2:      # q halves
                    t = ps.tile([128, 512], F32, tag="aux", bufs=1,
                                name=f"psq1_{half}")
                    for kc in range(2):
                        nc.tensor.matmul(t, wqp[kc][:, 128:256],
                                         xr[kc][:, csl],
                                         start=(kc == 0), stop=(kc == 1))
                    nc.vector.tensor_scalar(out=q_sb[1][:, csl], in0=t,
                                            scalar1=qb_cols[1],
                                            scalar2=None, op0=OP.add)
                else:          # k halves
                    t = ps.tile([128, 512], F32, tag="aux", bufs=1,
                                name=f"psk1_{half}")
                    for kc in range(2):
                        nc.tensor.matmul(t, wkp[kc][:, 128:256],
                                         xr[kc][:, csl],
                                         start=(kc == 0), stop=(kc == 1))
                    nc.vector.tensor_copy(k_sb[1][:, csl], t)

            def emit_v_chunk(i):
                psv = ps.tile([128, C], F32, tag="aux", bufs=1,
                              name=f"psv{i}")
                for kc in range(2):
                    nc.tensor.matmul(
                        psv, xr[kc][:, 128 * i:128 * (i + 1)],
                        wvp[kc], start=(kc == 0), stop=(kc == 1))
                vt = pool.tile([128, C], BF16, name=f"vt{i}")
                nc.vector.scalar_tensor_tensor(
                    out=vt, in0=psv, scalar=1.0, in1=vtext_b,
                    op0=OP.bypass, op1=OP.add)
                vtt[i] = vt

            def emit_scores(s):
                Hh, h, kcp = stages[s]
                m, r = h // 2, h % 2
                ss = ps.tile([128, HW], F32, tag="qk", bufs=2,
                             name=f"ss{s}")
                for t in range(2):
                    kc = 2 * kcp + t
                    nc.tensor.matmul(
                        ss[:, 512 * t:512 * (t + 1)],
                        k_sb[m][64 * r:64 * (r + 1),
                                128 * kc:128 * (kc + 1)],
                        q_sb[m][64 * r:64 * (r + 1),
                                512 * Hh:512 * (Hh + 1)],
                        start=True, stop=True,
                        tile_position=(64 * r, 0))
                return ss

            def emit_exp(s, ss):
                et = pool.tile([128, HW], BF16, tag="e", bufs=4,
                               name=f"e{s}")
                if s in EXP_DVE_STAGES:
                    nc.vector.tensor_scalar(
                        out=et.bitcast(I16), in0=ss,
                        scalar1=float(EXP_A), scalar2=float(EXP_B),
                        op0=OP.mult, op1=OP.add)
                else:
                    nc.scalar.activation(et, ss, AF.Exp, scale=0.125)
                e_all[s] = et

            def emit_av(s):
                Hh, h, kcp = stages[s]
                et = e_all.pop(s)
                if (Hh, 0) not in av:
                    for pr in range(2):
                        av[(Hh, pr)] = ps.tile(
                            [128, 512], F32, tag="av", bufs=2,
                            name=f"av{Hh}{pr}")
                first = kcp == 0
                last = kcp == 3
                for t in range(2):
                    for j in range(4):
                        lhs = et[:, 512 * t + 128 * j:
                                 512 * t + 128 * (j + 1)]
                        nc.tensor.matmul(
                            av[(Hh, j // 2)][:, 256 * (j % 2) + 64 * h:
                                             256 * (j % 2) + 64 * (h + 1)],
                            lhs, vtt[2 * kcp + t][:, 64 * h:64 * (h + 1)],
                            start=(first and t == 0),
                            stop=(last and t == 1))
                        nc.tensor.matmul(
                            den[:, 16 * Hh + 4 * j + h:
                                16 * Hh + 4 * j + h + 1],
                            lhs, ones_bf,
                            start=(first and t == 0),
                            stop=(last and t == 1))

            def emit_head_tail(s):
                Hh, h, _ = stages[s]
                rinv = pool.tile([128, 4], F32, tag="ri", bufs=4,
                                 name=f"ri{Hh}{h}")
                dsl = den.rearrange("p (g j hh) -> p g j hh",
                                    g=2, j=4, hh=4)[:, Hh, :, h]
                nc.vector.reciprocal_approx_fast(rinv, dsl)
                for j in range(4):
                    nc.vector.scalar_tensor_tensor(
                        out=ostage[Hh][:, 256 * j + 64 * h:
                                       256 * j + 64 * (h + 1)],
                        in0=av[(Hh, j // 2)][:, 256 * (j % 2) + 64 * h:
                                             256 * (j % 2) + 64 * (h + 1)],
                        scalar=rinv[:, j:j + 1],
                        in1=xT[:, 256 * (4 * Hh + j) + 64 * h:
                               256 * (4 * Hh + j) + 64 * (h + 1)],
                        op0=OP.mult, op1=OP.add)
                    if h == 3:
                        qi = 4 * Hh + j
                        nc.sync.dma_start(
                            out_d.ap()[128 * qi:128 * (qi + 1), :],
                            ostage[Hh][:, 256 * j:256 * (j + 1)])
                if h == 3:
                    for pr in range(2):
                        del av[(Hh, pr)]

            for s in range(len(stages)):
                ss = emit_scores(s)
                if s < 4:
                    emit_v_chunk(2 * s)
                    emit_v_chunk(2 * s + 1)
                    emit_m1_piece(s)
                emit_exp(s, ss)
                if s >= 1:
                    emit_av(s - 1)
                    if stages[s - 1][2] == 3:
                        emit_head_tail(s - 1)
            emit_av(31)
            emit_head_tail(31)

    nc.finalize()
    return nc


def _get_program():
    global _PROGRAM
    if _PROGRAM is None:
        _PROGRAM = _build_program()
    return _PROGRAM


def kernel(x, text_feat, gn_gamma, gn_beta, W0, b0, W1, b1, W2, b2):
    global _last_in_maps
    x = np.ascontiguousarray(np.asarray(x, dtype=np.float32))
    text_feat = np.ascontiguousarray(np.asarray(text_feat, dtype=np.float32))
    f32 = lambda a: np.ascontiguousarray(np.asarray(a, dtype=np.float32))
    W0, b0, W1, b1, W2, b2 = map(f32, (W0, b0, W1, b1, W2, b2))
    gn_gamma, gn_beta = f32(gn_gamma), f32(gn_beta)
    B = x.shape[0]

    gmat = np.zeros((CIN, NG), np.float32)
    for c in range(CIN):
        gmat[c, c // CPG] = 1.0 if c < C else float(HW)
    gmat_p = np.ascontiguousarray(
        gmat.reshape(6, 128, NG).transpose(1, 0, 2).reshape(128, 6 * NG))
    emat = np.zeros((NG, CIN), np.float32)
    for c in range(CIN):
        emat[c // CPG, c] = 1.0

    wall = np.empty((128, 1536), np.float32)
    for kc in range(2):
        for pi, W in enumerate((W0, W1, W2)):
            wall[:, 768 * kc + 256 * pi:768 * kc + 256 * (pi + 1)] = \
                W[:C][128 * kc:128 * (kc + 1), :]
    import ml_dtypes
    wtext = np.empty((128, 2048), ml_dtypes.bfloat16)
    for kc in range(4):
        wtext[:, 512 * kc:512 * kc + 256] = W0[C:][128 * kc:128 * (kc + 1), :]
        wtext[:, 512 * kc + 256:512 * kc + 512] = \
            W2[C:][128 * kc:128 * (kc + 1), :]

    shared = {
        "gmat": gmat_p, "emat": emat, "wall": wall, "wtext": wtext,
        "b2row": b2.reshape(1, C),
    }
    in_maps = []
    for b in range(B):
        misc = np.zeros((128, 18), np.float32)
        misc[:, 0:4] = text_feat[b].reshape(4, 128).T
        misc[:, 4:10] = gn_gamma.reshape(6, 128).T
        misc[:, 10:16] = gn_beta.reshape(6, 128).T
        misc[:, 16:18] = b0.reshape(2, 128).T
        m = dict(shared)
        xb = x[b].reshape(C, HW)
        m["x"] = np.ascontiguousarray(xb)
        m["xt"] = np.ascontiguousarray(xb.T)
        m["misc"] = misc
        in_maps.append(m)

    _last_in_maps = in_maps
    nc = _get_program()
    res = run_bass_kernel_spmd(nc, in_maps, core_ids=list(range(B)))
    out = np.stack([r["out"].reshape(HW, C).T.reshape(C, 32, 32)
                    for r in res.results])
    return np.ascontiguousarray(out).astype(np.float32)
